# revision 17
# baseline (speedup 1.0000x reference)
"""Trainium2 Bass kernel for nn_MetricSelfAttention.

Reference computation (B=4, W=2048, C=1024, N=16 heads, K=64):
    metric_n = P_n @ P_n^T                  (per-head bilinear form)
    proj = X @ W_proj^T ; split into per-head Q_n [W, K]
    S_n = tril(Q_n M_n Q_n^T) / sqrt(K)     (multiplicative causal mask, no softmax)
    U_n = S_n @ Q_n
    out = concat_n(U_n @ T_n) @ W_mixer^T

Device algorithm (per core; 8 cores = 4 batches x 2 head-groups of 8 heads):
  Host folds:  M'_n = P_n P_n^T / sqrt(K),  Wm2_n = T_n @ W_mixer[:, nK:(n+1)K]^T
  so that out_partial = sum_n U_n @ Wm2_n with U_n = tril(Q_n M'_n Q_n^T) @ Q_n.

  Causal decomposition (block row i of 128):
    U_i = A_i @ KV_i + tril(A_i Q_i^T) @ Q_i,   A = Q M',  KV_i = sum_{j<i} Q_j^T Q_j
  which turns the O(W^2 K) masked product into O(W K^2) prefix work plus
  O(W * 128 * K) diagonal blocks -- a ~4.5x FLOP reduction vs dense-causal.

  Heads are processed in pairs stacked on the partition dim (2x64=128) to keep
  the PE array full.  All matmuls run in bf16 with fp32 PSUM accumulation.
"""

import os
import sys

import numpy as np
import ml_dtypes

if "/opt/trn_rl_repo" not in sys.path:
    sys.path.insert(0, "/opt/trn_rl_repo")

import concourse.bass as bass
import concourse.tile as tile
from concourse import bacc, mybir
from concourse.bass_utils import run_bass_kernel_spmd

BF16 = mybir.dt.bfloat16
F32 = mybir.dt.float32

B, W, C, NHEADS, K = 4, 2048, 1024, 16, 64
HPG = 8          # heads per group (per core)
NPAIR = 4        # head pairs per core
GK = HPG * K     # 512: head-group projection width

_NC_CACHE = {}
LAST_RESULTS = None  # for test.py introspection (exec_time_ns etc.)


def build_nc(w=W, mm_dt=BF16):
    """Build the per-core Bass program. Parameterized by sequence length for
    small-scale simulator testing."""
    nw = w // 128           # number of 128-row w-tiles
    csub = C // 128         # 8 contraction subtiles for the projections
    chunk = min(512, w)
    nch = w // chunk        # 512-wide chunks of the sequence dim

    nc = bacc.Bacc()
    xt_d = nc.declare_dram_parameter("xt", [C, w], mm_dt, isOutput=False)
    wpt_d = nc.declare_dram_parameter("wpt", [C, GK], mm_dt, isOutput=False)
    mblk_d = nc.declare_dram_parameter("mblk", [NPAIR, 128, 128], mm_dt, isOutput=False)
    wm2_d = nc.declare_dram_parameter("wm2", [NPAIR, 128, C], mm_dt, isOutput=False)
    triu2_d = nc.declare_dram_parameter("triu2", [128, 256], F32, isOutput=False)
    blkd_d = nc.declare_dram_parameter("blkd", [128, 128], F32, isOutput=False)
    out_d = nc.declare_dram_parameter("out", [w, C], F32, isOutput=True)

    from contextlib import ExitStack

    with tile.TileContext(nc) as tc, ExitStack() as ctx:
        const = ctx.enter_context(tc.tile_pool(name="const", bufs=1))
        persist = ctx.enter_context(tc.tile_pool(name="persist", bufs=1))

        # ---- constant / input loads ----
        xt_sb = []
        for s in range(csub):
            t = const.tile([128, w], mm_dt, name=f"xt{s}", tag=f"xt{s}")
            nc.sync.dma_start(t[:], xt_d[128 * s:128 * (s + 1), :])
            xt_sb.append(t)
        wpt_sb = []
        for s in range(csub):
            t = const.tile([128, GK], mm_dt, name=f"wpt{s}", tag=f"wpt{s}")
            nc.sync.dma_start(t[:], wpt_d[128 * s:128 * (s + 1), :])
            wpt_sb.append(t)
        mblk_sb = const.tile([128, NPAIR * 128], mm_dt, name="mblk", tag="mblk")
        for p in range(NPAIR):
            nc.sync.dma_start(mblk_sb[:, 128 * p:128 * (p + 1)], mblk_d[p])
        wm2_sb = []
        for p in range(NPAIR):
            t = const.tile([128, C], mm_dt, name=f"wm2_{p}", tag=f"wm2_{p}")
            nc.sync.dma_start(t[:], wm2_d[p])
            wm2_sb.append(t)
        triu2_sb = const.tile([128, 256], F32, name="triu2", tag="triu2")
        nc.sync.dma_start(triu2_sb[:], triu2_d[:])
        blkd_sb = const.tile([128, 128], F32, name="blkd", tag="blkd")
        nc.sync.dma_start(blkd_sb[:], blkd_d[:])

        # ---- persistent intermediates (bf16) ----
        # q_nat: natural layout [w, k] -- w-tile i occupies cols [512i, 512i+512),
        #        inside which head h (0..7) owns cols [64h, 64h+64).
        q_nat = persist.tile([128, nw * GK], mm_dt, name="q_nat", tag="q_nat")
        # qt/at: transposed layout per pair p: cols [p*w, (p+1)*w); partitions
        #        0-63 = head 2p's K dims, 64-127 = head 2p+1's.
        qt_sb = persist.tile([128, NPAIR * w], mm_dt, name="qt_sb", tag="qt_sb")
        at_sb = persist.tile([128, NPAIR * w], mm_dt, name="at_sb", tag="at_sb")

        # ---- phase A: natural projection  Q[wtile] = XT[:,wtile]^T @ WpT ----
        with tc.tile_pool(name="psA", bufs=4, space="PSUM") as psA:
            for i in range(nw):
                ps = psA.tile([128, GK], F32, name="projnat", tag="projnat")
                for s in range(csub):
                    nc.tensor.matmul(
                        ps[:],
                        lhsT=xt_sb[s][:, 128 * i:128 * (i + 1)],
                        rhs=wpt_sb[s][:],
                        start=(s == 0),
                        stop=(s == csub - 1),
                    )
                nc.vector.tensor_copy(q_nat[:, GK * i:GK * (i + 1)], ps[:])

        # collapse cross-phase dep fan-in (walrus: "Too many sync wait
        # commands" on the first post-phase matmul otherwise)
        tc.strict_bb_all_engine_barrier()

        # ---- phase B: transposed projection per pair:
        #      QT_pair[:, chunk] = WpT[:, pair]^T @ XT[:, chunk] ----
        with tc.tile_pool(name="psB", bufs=4, space="PSUM") as psB:
            for p in range(NPAIR):
                for ch in range(nch):
                    ps = psB.tile([128, chunk], F32, name="qtps", tag="qtps")
                    for s in range(csub):
                        nc.tensor.matmul(
                            ps[:],
                            lhsT=wpt_sb[s][:, 128 * p:128 * (p + 1)],
                            rhs=xt_sb[s][:, chunk * ch:chunk * (ch + 1)],
                            start=(s == 0),
                            stop=(s == csub - 1),
                        )
                    nc.vector.tensor_copy(
                        qt_sb[:, p * w + chunk * ch:p * w + chunk * (ch + 1)], ps[:]
                    )
            # ---- phase C: AT_pair = blockdiag(M'a, M'b) @ QT_pair ----
            for p in range(NPAIR):
                for ch in range(nch):
                    ps = psB.tile([128, chunk], F32, name="atps", tag="qtps")
                    nc.tensor.matmul(
                        ps[:],
                        lhsT=mblk_sb[:, 128 * p:128 * (p + 1)],
                        rhs=qt_sb[:, p * w + chunk * ch:p * w + chunk * (ch + 1)],
                        start=True,
                        stop=True,
                    )
                    nc.vector.tensor_copy(
                        at_sb[:, p * w + chunk * ch:p * w + chunk * (ch + 1)], ps[:]
                    )

        tc.strict_bb_all_engine_barrier()

        # ---- phase D: causal attention + mixer, one 128-row w-tile at a time ----
        gterm_pool = ctx.enter_context(tc.tile_pool(name="gterm", bufs=1, space="PSUM"))
        st_pool = ctx.enter_context(tc.tile_pool(name="stp", bufs=3, space="PSUM"))
        ut_pool = ctx.enter_context(tc.tile_pool(name="utp", bufs=2, space="PSUM"))
        mix_pool = ctx.enter_context(tc.tile_pool(name="mixp", bufs=2, space="PSUM"))
        work = ctx.enter_context(tc.tile_pool(name="work", bufs=3))

        # SBUF fp32 accumulator for the prefix Gram sum_j Q_j^T Q_j
        # (pair p occupies cols 128p..128p+128).
        gram_sb = persist.tile([128, NPAIR * 128], F32, name="gram_sb", tag="gram_sb")
        nc.vector.memset(gram_sb[:], 0.0)

        for i in range(nw):
            # 1) KV_i snapshot = blockdiag-masked prefix Gram (strictly j < i)
            kvs = []
            if i > 0:
                for p in range(NPAIR):
                    kv = work.tile([128, 128], mm_dt, name=f"kv{p}", tag=f"kv{p}",
                                   bufs=2)
                    nc.vector.tensor_mul(
                        kv[:], gram_sb[:, 128 * p:128 * (p + 1)], blkd_sb[:]
                    )
                    kvs.append(kv)

            # 2) Gram update with block j=i (after the snapshot)
            gterm = gterm_pool.tile([128, NPAIR * 128], F32, name="gterm", tag="gterm")
            for p in range(NPAIR):
                qp = q_nat[:, GK * i + 128 * p:GK * i + 128 * (p + 1)]
                nc.tensor.matmul(
                    gterm[:, 128 * p:128 * (p + 1)],
                    lhsT=qp,
                    rhs=qp,
                    start=(p == 0),
                    stop=(p == NPAIR - 1),
                )
            nc.vector.tensor_add(gram_sb[:], gram_sb[:], gterm[:])

            # 3) diagonal blocks S_ii^T = Q_i @ A_i^T per head (row-tiled pairs;
            #    the two concurrent row-group matmuls MUST hit different PSUM
            #    banks -- same-bank concurrent PE writes crash the device),
            #    masked with triu (incl diag) while copying to SBUF.
            st_sbs = []
            for p in range(NPAIR):
                st_pair = []
                for h in range(2):
                    stp = st_pool.tile([128, 128], F32, name="st", tag="st")
                    nc.tensor.matmul(
                        stp[:],
                        lhsT=qt_sb[64 * h:64 * (h + 1), p * w + 128 * i:p * w + 128 * (i + 1)],
                        rhs=at_sb[64 * h:64 * (h + 1), p * w + 128 * i:p * w + 128 * (i + 1)],
                        start=True,
                        stop=True,
                    )
                    st_pair.append(stp)
                st_sb = work.tile([128, 256], mm_dt, name="st_sb", tag="st_sb", bufs=3)
                for h in range(2):
                    nc.vector.tensor_mul(
                        st_sb[:, 128 * h:128 * (h + 1)], st_pair[h][:],
                        triu2_sb[:, 0:128],
                    )
                st_sbs.append(st_sb)

            # 4) UT_pair_i [128(k-pair), 128(w)] = KV_i^T A_i^T + Q_i^T Smask_ii^T
            ut_sbs = []
            for p in range(NPAIR):
                utp = ut_pool.tile([128, 128], F32, name="ut", tag="ut")
                if i > 0:
                    nc.tensor.matmul(
                        utp[:],
                        lhsT=kvs[p][:],
                        rhs=at_sb[:, p * w + 128 * i:p * w + 128 * (i + 1)],
                        start=True,
                        stop=False,
                        skip_group_check=True,
                    )
                for h in range(2):
                    # partition-split accumulation group: the sim's flat
                    # zero-region bookkeeping can't express it (HW has_written
                    # bits are per partition), so skip the sim-side check
                    nc.tensor.matmul(
                        utp[64 * h:64 * (h + 1), :],
                        lhsT=q_nat[:, GK * i + 128 * p + 64 * h:GK * i + 128 * p + 64 * (h + 1)],
                        rhs=st_sbs[p][:, 128 * h:128 * (h + 1)],
                        start=(i == 0),
                        stop=True,
                        skip_group_check=True,
                    )
                ut_sb = work.tile([128, 128], mm_dt, name="ut_sb", tag="ut_sb", bufs=6)
                nc.vector.tensor_copy(ut_sb[:], utp[:])
                ut_sbs.append(ut_sb)

            # 5) mixer: out[i-block] = sum_p UT_pair_i^T @ Wm2_pair
            out_sb = work.tile([128, C], F32, name="out_sb", tag="out_sb", bufs=2)
            for cm in range(C // 512):
                mx = mix_pool.tile([128, 512], F32, name="mx", tag="mx")
                for p in range(NPAIR):
                    nc.tensor.matmul(
                        mx[:],
                        lhsT=ut_sbs[p][:],
                        rhs=wm2_sb[p][:, 512 * cm:512 * (cm + 1)],
                        start=(p == 0),
                        stop=(p == NPAIR - 1),
                    )
                nc.vector.tensor_copy(out_sb[:, 512 * cm:512 * (cm + 1)], mx[:])
            nc.sync.dma_start(out_d[128 * i:128 * (i + 1), :], out_sb[:])

    # Bacc defers register allocation + wait-splitting to finalize();
    # run_bass_via_pjrt serializes the module as-is, so finalize here.
    nc.finalize()
    return nc


def _get_nc(w=W):
    if w not in _NC_CACHE:
        _NC_CACHE[w] = build_nc(w)
    return _NC_CACHE[w]


def make_in_maps(x, wp, pm, tf, wm, w=W):
    """Host-side shard prep: per-core input dict list (cores c: b=c%4, g=c//4)."""
    bf = ml_dtypes.bfloat16
    metric = np.einsum("nij,nkj->nik", pm, pm) / np.sqrt(np.float32(K))
    # Wm2_n = T_n @ W_mixer[:, nK:(n+1)K]^T : [K, C]
    wm2 = np.stack([tf[n] @ wm[:, n * K:(n + 1) * K].T for n in range(NHEADS)])

    triu2 = np.zeros((128, 256), np.float32)
    tri = np.triu(np.ones((128, 128), np.float32))
    triu2[:, :128] = tri
    triu2[:, 128:] = tri
    blkd = np.zeros((128, 128), np.float32)
    blkd[:64, :64] = 1.0
    blkd[64:, 64:] = 1.0

    in_maps = []
    for c in range(8):
        b, g = c % 4, c // 4
        xt = np.ascontiguousarray(x[b][:w].T).astype(bf)                    # [C, w]
        wpt = np.ascontiguousarray(wp[GK * g:GK * (g + 1), :].T).astype(bf)  # [C, GK]
        mblk = np.zeros((NPAIR, 128, 128), np.float32)
        wm2c = np.zeros((NPAIR, 128, C), np.float32)
        for p in range(NPAIR):
            ha, hb = HPG * g + 2 * p, HPG * g + 2 * p + 1
            mblk[p, :64, :64] = metric[ha]
            mblk[p, 64:, 64:] = metric[hb]
            wm2c[p, :64, :] = wm2[ha]
            wm2c[p, 64:, :] = wm2[hb]
        in_maps.append({
            "xt": xt,
            "wpt": wpt,
            "mblk": mblk.astype(bf),
            "wm2": wm2c.astype(bf),
            "triu2": triu2,
            "blkd": blkd,
        })
    return in_maps


def _ensure_ntff_hook():
    """The agent image lacks antenv.axon_hooks; synthesize it and register the
    ctypes NTFF profile hook from trn_agent_boot so trace=True works."""
    try:
        from antenv.axon_hooks import get_axon_ntff_profile_hook  # noqa: F401
        return
    except ImportError:
        pass
    import types

    import antenv

    mod = types.ModuleType("antenv.axon_hooks")
    _box = {}
    mod.set_axon_ntff_profile_hook = lambda h: _box.__setitem__("h", h)
    mod.get_axon_ntff_profile_hook = lambda: _box.get("h")
    sys.modules["antenv.axon_hooks"] = mod
    antenv.axon_hooks = mod
    try:
        from trn_agent_boot.trn_boot import _ntff_profile_via_ctypes

        h = _ntff_profile_via_ctypes("/opt/axon/libaxon_pjrt.so")
        if h is not None:
            mod.set_axon_ntff_profile_hook(h)
    except Exception as e:  # profiling degrades, run still works
        print(f"ntff hook setup failed: {e}", file=sys.stderr)


def kernel(**inputs):
    global LAST_RESULTS
    x = np.asarray(inputs["in_sequence_bwc"], np.float32)
    wp = np.asarray(inputs["W_proj"], np.float32)
    pm = np.asarray(inputs["pre_metric_nkk"], np.float32)
    tf = np.asarray(inputs["transforms_nkk"], np.float32)
    wm = np.asarray(inputs["W_mixer"], np.float32)

    in_maps = make_in_maps(x, wp, pm, tf, wm)
    nc = _get_nc()
    trace = bool(int(os.environ.get("KERNEL_TRACE", "0")))
    if trace:
        _ensure_ntff_hook()
    res = run_bass_kernel_spmd(nc, in_maps, list(range(8)), trace=trace)
    LAST_RESULTS = res
    outs = [r["out"] for r in res.results]
    full = np.empty((B, W, C), np.float32)
    for b in range(B):
        full[b] = outs[b] + outs[4 + b]
    return full


# revision 18
# speedup vs baseline: 1.0506x; 1.0506x over previous
"""Trainium2 Bass kernel for nn_MetricSelfAttention.

Reference computation (B=4, W=2048, C=1024, N=16 heads, K=64):
    metric_n = P_n @ P_n^T                  (per-head bilinear form)
    proj = X @ W_proj^T ; split into per-head Q_n [W, K]
    S_n = tril(Q_n M_n Q_n^T) / sqrt(K)     (multiplicative causal mask, no softmax)
    U_n = S_n @ Q_n
    out = concat_n(U_n @ T_n) @ W_mixer^T

Device algorithm (per core; 8 cores = 4 batches x 2 head-groups of 8 heads):
  Host folds:  M'_n = P_n P_n^T / sqrt(K),  Wm2_n = T_n @ W_mixer[:, nK:(n+1)K]^T
  so that out_partial = sum_n U_n @ Wm2_n with U_n = tril(Q_n M'_n Q_n^T) @ Q_n.

  Causal decomposition (block row i of 128):
    U_i = A_i @ KV_i + tril(A_i Q_i^T) @ Q_i,   A = Q M',  KV_i = sum_{j<i} Q_j^T Q_j
  which turns the O(W^2 K) masked product into O(W K^2) prefix work plus
  O(W * 128 * K) diagonal blocks -- a ~4.5x FLOP reduction vs dense-causal.

  Heads are processed in pairs stacked on the partition dim (2x64=128) to keep
  the PE array full.  All matmuls run in bf16 with fp32 PSUM accumulation.
"""

import os
import sys

import numpy as np
import ml_dtypes

if "/opt/trn_rl_repo" not in sys.path:
    sys.path.insert(0, "/opt/trn_rl_repo")

import concourse.bass as bass
import concourse.tile as tile
from concourse import bacc, mybir
from concourse.bass_utils import run_bass_kernel_spmd

BF16 = mybir.dt.bfloat16
F32 = mybir.dt.float32

B, W, C, NHEADS, K = 4, 2048, 1024, 16, 64
HPG = 8          # heads per group (per core)
NPAIR = 4        # head pairs per core
GK = HPG * K     # 512: head-group projection width

_NC_CACHE = {}
LAST_RESULTS = None  # for test.py introspection (exec_time_ns etc.)


def build_nc(w=W, mm_dt=BF16):
    """Build the per-core Bass program. Parameterized by sequence length for
    small-scale simulator testing."""
    nw = w // 128           # number of 128-row w-tiles
    csub = C // 128         # 8 contraction subtiles for the projections
    chunk = min(512, w)
    nch = w // chunk        # 512-wide chunks of the sequence dim

    nc = bacc.Bacc()
    xt_d = nc.declare_dram_parameter("xt", [C, w], mm_dt, isOutput=False)
    wpt_d = nc.declare_dram_parameter("wpt", [C, GK], mm_dt, isOutput=False)
    mblk_d = nc.declare_dram_parameter("mblk", [NPAIR, 128, 128], mm_dt, isOutput=False)
    wm2_d = nc.declare_dram_parameter("wm2", [NPAIR, 128, C], mm_dt, isOutput=False)
    triu2_d = nc.declare_dram_parameter("triu2", [128, 256], F32, isOutput=False)
    blkd_d = nc.declare_dram_parameter("blkd", [128, 128], F32, isOutput=False)
    out_d = nc.declare_dram_parameter("out", [w, C], F32, isOutput=True)

    from contextlib import ExitStack

    with tile.TileContext(nc) as tc, ExitStack() as ctx:
        const = ctx.enter_context(tc.tile_pool(name="const", bufs=1))
        persist = ctx.enter_context(tc.tile_pool(name="persist", bufs=1))

        # ---- constant / input loads ----
        # xt/wpt are loaded chunk-major so phase A's first matmuls only wait
        # for the first w-chunk of every c-subtile, not the whole 5MB.
        wpt_sb = []
        for s in range(csub):
            t = const.tile([128, GK], mm_dt, name=f"wpt{s}", tag=f"wpt{s}")
            nc.sync.dma_start(t[:], wpt_d[128 * s:128 * (s + 1), :])
            wpt_sb.append(t)
        xt_sb = [const.tile([128, w], mm_dt, name=f"xt{s}", tag=f"xt{s}")
                 for s in range(csub)]
        for ch in range(nch):
            for s in range(csub):
                nc.sync.dma_start(
                    xt_sb[s][:, chunk * ch:chunk * (ch + 1)],
                    xt_d[128 * s:128 * (s + 1), chunk * ch:chunk * (ch + 1)],
                )
        mblk_sb = const.tile([128, NPAIR * 128], mm_dt, name="mblk", tag="mblk")
        for p in range(NPAIR):
            nc.sync.dma_start(mblk_sb[:, 128 * p:128 * (p + 1)], mblk_d[p])
        wm2_sb = []
        for p in range(NPAIR):
            t = const.tile([128, C], mm_dt, name=f"wm2_{p}", tag=f"wm2_{p}")
            nc.sync.dma_start(t[:], wm2_d[p])
            wm2_sb.append(t)
        triu2_sb = const.tile([128, 256], F32, name="triu2", tag="triu2")
        nc.sync.dma_start(triu2_sb[:], triu2_d[:])
        blkd_sb = const.tile([128, 128], F32, name="blkd", tag="blkd")
        nc.sync.dma_start(blkd_sb[:], blkd_d[:])

        # ---- persistent intermediates (bf16) ----
        # q_nat: natural layout [w, k] -- w-tile i occupies cols [512i, 512i+512),
        #        inside which head h (0..7) owns cols [64h, 64h+64).
        q_nat = persist.tile([128, nw * GK], mm_dt, name="q_nat", tag="q_nat")
        # qt/at: transposed layout per pair p: cols [p*w, (p+1)*w); partitions
        #        0-63 = head 2p's K dims, 64-127 = head 2p+1's.
        qt_sb = persist.tile([128, NPAIR * w], mm_dt, name="qt_sb", tag="qt_sb")
        at_sb = persist.tile([128, NPAIR * w], mm_dt, name="at_sb", tag="at_sb")
        # per-i blockdiag(KV_a, KV_b) lhsT tiles for the U-main matmuls
        kv_sb = persist.tile([128, NPAIR * nw * 128], mm_dt, name="kv_sb", tag="kv_sb")

        # ---- phase A: natural projection  Q[wtile] = XT[:,wtile]^T @ WpT ----
        with tc.tile_pool(name="psA", bufs=4, space="PSUM") as psA:
            for i in range(nw):
                ps = psA.tile([128, GK], F32, name="projnat", tag="projnat")
                for s in range(csub):
                    nc.tensor.matmul(
                        ps[:],
                        lhsT=xt_sb[s][:, 128 * i:128 * (i + 1)],
                        rhs=wpt_sb[s][:],
                        start=(s == 0),
                        stop=(s == csub - 1),
                    )
                nc.vector.tensor_copy(q_nat[:, GK * i:GK * (i + 1)], ps[:])

            # ---- phase B: transposed projection per pair:
            #      QT_pair[:, chunk] = WpT[:, pair]^T @ XT[:, chunk] ----
            for p in range(NPAIR):
                for ch in range(nch):
                    ps = psA.tile([128, chunk], F32, name="qtps", tag="projnat")
                    for s in range(csub):
                        nc.tensor.matmul(
                            ps[:],
                            lhsT=wpt_sb[s][:, 128 * p:128 * (p + 1)],
                            rhs=xt_sb[s][:, chunk * ch:chunk * (ch + 1)],
                            start=(s == 0),
                            stop=(s == csub - 1),
                        )
                    nc.vector.tensor_copy(
                        qt_sb[:, p * w + chunk * ch:p * w + chunk * (ch + 1)], ps[:]
                    )
            # ---- phase C: AT_pair = blockdiag(M'a, M'b) @ QT_pair ----
            for p in range(NPAIR):
                for ch in range(nch):
                    ps = psA.tile([128, chunk], F32, name="atps", tag="projnat")
                    nc.tensor.matmul(
                        ps[:],
                        lhsT=mblk_sb[:, 128 * p:128 * (p + 1)],
                        rhs=qt_sb[:, p * w + chunk * ch:p * w + chunk * (ch + 1)],
                        start=True,
                        stop=True,
                    )
                    nc.vector.tensor_copy(
                        at_sb[:, p * w + chunk * ch:p * w + chunk * (ch + 1)], ps[:]
                    )

            # ---- phase D0: Gram prefix.  For each i: snapshot the blockdiag-
            # masked prefix (strictly j<i) into kv_sb, then accumulate block i.
            # PE (gram terms) pipelines ahead of the DVE adds / GpSimd masks.
            gram_sb = persist.tile([128, NPAIR * 128], F32, name="gram_sb",
                                   tag="gram_sb")
            nc.vector.memset(gram_sb[:], 0.0)
            for i in range(nw):
                if i > 0:
                    for p in range(NPAIR):
                        nc.gpsimd.tensor_mul(
                            kv_sb[:, (p * nw + i) * 128:(p * nw + i) * 128 + 128],
                            gram_sb[:, 128 * p:128 * (p + 1)], blkd_sb[:],
                        )
                if i < nw - 1:  # last block's gram term is never consumed
                    gterm = psA.tile([128, NPAIR * 128], F32, name="gterm",
                                     tag="projnat")
                    for p in range(NPAIR):
                        qp = q_nat[:, GK * i + 128 * p:GK * i + 128 * (p + 1)]
                        nc.tensor.matmul(
                            gterm[:, 128 * p:128 * (p + 1)],
                            lhsT=qp, rhs=qp,
                            start=(p == 0),
                            stop=(p == NPAIR - 1),
                        )
                    nc.vector.tensor_add(gram_sb[:], gram_sb[:], gterm[:])

        # ---- phase D1: diagonal blocks + U assembly + mixer, per w-tile ----
        st_pool = ctx.enter_context(tc.tile_pool(name="stp", bufs=3, space="PSUM"))
        ut_pool = ctx.enter_context(tc.tile_pool(name="utp", bufs=3, space="PSUM"))
        mix_pool = ctx.enter_context(tc.tile_pool(name="mixp", bufs=2, space="PSUM"))
        work = ctx.enter_context(tc.tile_pool(name="work", bufs=3))

        for i in range(nw):
            # diagonal blocks S_ii^T = Q_i @ A_i^T per head (row-tiled pairs;
            # the two concurrent row-group matmuls MUST hit different PSUM
            # banks -- same-bank concurrent PE writes crash the device),
            # masked with triu (incl diag) while copying to SBUF.
            st_sbs = []
            for p in range(NPAIR):
                st_pair = []
                for h in range(2):
                    stp = st_pool.tile([128, 128], F32, name="st", tag="st")
                    nc.tensor.matmul(
                        stp[:],
                        lhsT=qt_sb[64 * h:64 * (h + 1), p * w + 128 * i:p * w + 128 * (i + 1)],
                        rhs=at_sb[64 * h:64 * (h + 1), p * w + 128 * i:p * w + 128 * (i + 1)],
                        start=True,
                        stop=True,
                    )
                    st_pair.append(stp)
                st_sb = work.tile([128, 256], mm_dt, name="st_sb", tag="st_sb", bufs=3)
                for h in range(2):
                    nc.vector.tensor_mul(
                        st_sb[:, 128 * h:128 * (h + 1)], st_pair[h][:],
                        triu2_sb[:, 0:128],
                    )
                st_sbs.append(st_sb)

            # UT_pair_i [128(k-pair), 128(w)] = KV_i^T A_i^T + Q_i^T Smask_ii^T
            ut_sbs = []
            for p in range(NPAIR):
                utp = ut_pool.tile([128, 128], F32, name="ut", tag="ut")
                if i > 0:
                    nc.tensor.matmul(
                        utp[:],
                        lhsT=kv_sb[:, (p * nw + i) * 128:(p * nw + i) * 128 + 128],
                        rhs=at_sb[:, p * w + 128 * i:p * w + 128 * (i + 1)],
                        start=True,
                        stop=False,
                        skip_group_check=True,
                    )
                for h in range(2):
                    # partition-split accumulation group: the sim's flat
                    # zero-region bookkeeping can't express it (HW has_written
                    # bits are per partition), so skip the sim-side check
                    nc.tensor.matmul(
                        utp[64 * h:64 * (h + 1), :],
                        lhsT=q_nat[:, GK * i + 128 * p + 64 * h:GK * i + 128 * p + 64 * (h + 1)],
                        rhs=st_sbs[p][:, 128 * h:128 * (h + 1)],
                        start=(i == 0),
                        stop=True,
                        skip_group_check=True,
                    )
                ut_sb = work.tile([128, 128], mm_dt, name="ut_sb", tag="ut_sb", bufs=6)
                nc.scalar.copy(ut_sb[:], utp[:])
                ut_sbs.append(ut_sb)

            # mixer: out[i-block] = sum_p UT_pair_i^T @ Wm2_pair
            out_sb = work.tile([128, C], F32, name="out_sb", tag="out_sb", bufs=2)
            for cm in range(C // 512):
                mx = mix_pool.tile([128, 512], F32, name="mx", tag="mx")
                for p in range(NPAIR):
                    nc.tensor.matmul(
                        mx[:],
                        lhsT=ut_sbs[p][:],
                        rhs=wm2_sb[p][:, 512 * cm:512 * (cm + 1)],
                        start=(p == 0),
                        stop=(p == NPAIR - 1),
                    )
                nc.scalar.copy(out_sb[:, 512 * cm:512 * (cm + 1)], mx[:])
            nc.sync.dma_start(out_d[128 * i:128 * (i + 1), :], out_sb[:])

    # Bacc defers register allocation + wait-splitting to finalize();
    # run_bass_via_pjrt serializes the module as-is, so finalize here.
    nc.finalize()
    return nc


def _get_nc(w=W):
    if w not in _NC_CACHE:
        _NC_CACHE[w] = build_nc(w)
    return _NC_CACHE[w]


def make_in_maps(x, wp, pm, tf, wm, w=W):
    """Host-side shard prep: per-core input dict list (cores c: b=c%4, g=c//4)."""
    bf = ml_dtypes.bfloat16
    metric = np.einsum("nij,nkj->nik", pm, pm) / np.sqrt(np.float32(K))
    # Wm2_n = T_n @ W_mixer[:, nK:(n+1)K]^T : [K, C]
    wm2 = np.stack([tf[n] @ wm[:, n * K:(n + 1) * K].T for n in range(NHEADS)])

    triu2 = np.zeros((128, 256), np.float32)
    tri = np.triu(np.ones((128, 128), np.float32))
    triu2[:, :128] = tri
    triu2[:, 128:] = tri
    blkd = np.zeros((128, 128), np.float32)
    blkd[:64, :64] = 1.0
    blkd[64:, 64:] = 1.0

    in_maps = []
    for c in range(8):
        b, g = c % 4, c // 4
        xt = np.ascontiguousarray(x[b][:w].T).astype(bf)                    # [C, w]
        wpt = np.ascontiguousarray(wp[GK * g:GK * (g + 1), :].T).astype(bf)  # [C, GK]
        mblk = np.zeros((NPAIR, 128, 128), np.float32)
        wm2c = np.zeros((NPAIR, 128, C), np.float32)
        for p in range(NPAIR):
            ha, hb = HPG * g + 2 * p, HPG * g + 2 * p + 1
            mblk[p, :64, :64] = metric[ha]
            mblk[p, 64:, 64:] = metric[hb]
            wm2c[p, :64, :] = wm2[ha]
            wm2c[p, 64:, :] = wm2[hb]
        in_maps.append({
            "xt": xt,
            "wpt": wpt,
            "mblk": mblk.astype(bf),
            "wm2": wm2c.astype(bf),
            "triu2": triu2,
            "blkd": blkd,
        })
    return in_maps


def _ensure_ntff_hook():
    """The agent image lacks antenv.axon_hooks; synthesize it and register the
    ctypes NTFF profile hook from trn_agent_boot so trace=True works."""
    try:
        from antenv.axon_hooks import get_axon_ntff_profile_hook  # noqa: F401
        return
    except ImportError:
        pass
    import types

    import antenv

    mod = types.ModuleType("antenv.axon_hooks")
    _box = {}
    mod.set_axon_ntff_profile_hook = lambda h: _box.__setitem__("h", h)
    mod.get_axon_ntff_profile_hook = lambda: _box.get("h")
    sys.modules["antenv.axon_hooks"] = mod
    antenv.axon_hooks = mod
    try:
        from trn_agent_boot.trn_boot import _ntff_profile_via_ctypes

        h = _ntff_profile_via_ctypes("/opt/axon/libaxon_pjrt.so")
        if h is not None:
            mod.set_axon_ntff_profile_hook(h)
    except Exception as e:  # profiling degrades, run still works
        print(f"ntff hook setup failed: {e}", file=sys.stderr)


def kernel(**inputs):
    global LAST_RESULTS
    x = np.asarray(inputs["in_sequence_bwc"], np.float32)
    wp = np.asarray(inputs["W_proj"], np.float32)
    pm = np.asarray(inputs["pre_metric_nkk"], np.float32)
    tf = np.asarray(inputs["transforms_nkk"], np.float32)
    wm = np.asarray(inputs["W_mixer"], np.float32)

    in_maps = make_in_maps(x, wp, pm, tf, wm)
    nc = _get_nc()
    trace = bool(int(os.environ.get("KERNEL_TRACE", "0")))
    if trace:
        _ensure_ntff_hook()
    res = run_bass_kernel_spmd(nc, in_maps, list(range(8)), trace=trace)
    LAST_RESULTS = res
    outs = [r["out"] for r in res.results]
    full = np.empty((B, W, C), np.float32)
    for b in range(B):
        full[b] = outs[b] + outs[4 + b]
    return full
